# revision 1
# baseline (speedup 1.0000x reference)
"""Trainium2 Bass kernel for RecursiveMamba130M.

Math: the complex SSM state never needs materializing. With
  R = cos(theta) + j sin(theta),  Bc = Br + j Bi,  Cc = Cr + j Ci,
the per-loop output collapses to
  y_i[t, f] = sum_{k<=i} G_{i-k}[f] * u_k[t, f],   u_k = h_k @ W_in^T
where G_m[f] = sum_s Re(Cc * R^m * Bc)
            = sum_s (CrBr - CiBi) cos(m th) - (CrBi + CiBr) sin(m th).

Sharding: fully data-parallel over the 1024 sequence positions
(128 tokens per core, no collectives); small weights replicated.

Per-core device program (tokens on partitions, fp32/fp32r):
  loop i in 0..3:
    hT   = PE-transpose(h)                  (6x 128x128)
    u    = h @ W_in^T                       (PE, fp32r, N=512 tiles)
    y    = G0*u + acc_i ; acc_j += G_{j-i}*u  (DVE/Pool, G broadcast tiles)
    yT   = PE-transpose(y)                  (12x 128x128)
    z    = y @ out_proj^T                   (PE, fp32r)
    out  = rmsnorm(z); w = h + out; x' = rmsnorm(w); h = x' + step_emb[i+1]
  (norm sums via ACT Square+accum and the identity
   sum w^2 = rs_z^2*sum z^2 + 2 rs_z*sum z*h + sum h^2)
"""

import numpy as np

import concourse.bass as bass
import concourse.tile as tile
from concourse.bacc import Bacc
from concourse import masks, mybir
from concourse.bass_utils import run_bass_kernel_spmd

T = 128          # tokens per core
D = 768          # d_model
F = 1536         # 2 * d_model
NL = 4           # reasoning loops
NCORES = 8
EPS = 1e-6

f32 = mybir.dt.float32
f32r = mybir.dt.float32r
AL = mybir.AluOpType
AF = mybir.ActivationFunctionType

_CACHE = {}


def build_nc():
    nc = Bacc()
    x_d = nc.dram_tensor("x_in", [T, D], f32, kind="ExternalInput")
    winT_d = nc.dram_tensor("winT", [D, F], f32, kind="ExternalInput")
    woutT_d = nc.dram_tensor("woutT", [F, D], f32, kind="ExternalInput")
    g4_d = nc.dram_tensor("g4", [NL, F], f32, kind="ExternalInput")
    s4_d = nc.dram_tensor("s4", [NL, D], f32, kind="ExternalInput")
    out_d = nc.dram_tensor("x_out", [T, D], f32, kind="ExternalOutput")

    with tile.TileContext(nc) as tc:
        with (
            tc.tile_pool(name="wpool", bufs=1) as wpool,
            tc.tile_pool(name="apool", bufs=1) as apool,
            tc.tile_pool(name="work", bufs=2) as work,
            tc.tile_pool(name="scal", bufs=1) as scal,
            tc.tile_pool(name="ps_t", bufs=1, space="PSUM") as ps_t,
            tc.tile_pool(name="ps_u", bufs=1, space="PSUM") as ps_u,
            tc.tile_pool(name="ps_z", bufs=1, space="PSUM") as ps_z,
        ):
            # ---------- constants / weights ----------
            ident = wpool.tile([128, 128], f32, tag="ident")
            masks.make_identity(nc, ident[:])
            ones1 = wpool.tile([1, 128], f32r, tag="ones1")
            nc.vector.memset(ones1[:].bitcast(mybir.dt.uint32), 0x3F800000)
            eps_t = wpool.tile([T, 1], f32, tag="eps_t")
            nc.vector.memset(eps_t[:], EPS)

            x_sb = wpool.tile([T, D], f32, tag="x_sb")
            nc.sync.dma_start(x_sb[:], x_d[:, :])

            winT_sb = []
            for k in range(6):
                wt = wpool.tile([128, F], f32r, tag=f"winT{k}")
                nc.sync.dma_start(wt[:], winT_d[128 * k:128 * (k + 1), :].bitcast(f32r))
                winT_sb.append(wt)

            # step_emb broadcast tiles [128, D] via K=1 matmul
            Sb = []
            for i in range(NL):
                sr = work.tile([1, D], f32r, tag="s_row", bufs=2, name=f"s_row{i}")
                nc.sync.dma_start(sr[:], s4_d[i:i + 1, :].bitcast(f32r))
                sb_ps = ps_z.tile([T, D], f32, tag="z")
                for off, nn in ((0, 512), (512, 256)):
                    nc.tensor.matmul(
                        sb_ps[:, off:off + nn],
                        ones1[:, :],
                        sr[:, off:off + nn],
                        start=True, stop=True,
                    )
                sb = wpool.tile([T, D], f32, tag=f"Sb{i}")
                nc.scalar.copy(sb[:], sb_ps[:])
                Sb.append(sb)

            # G broadcast tiles [128, F]
            Gb = []
            for m in range(NL):
                gr = work.tile([1, F], f32r, tag="g_row", bufs=2, name=f"g_row{m}")
                nc.sync.dma_start(gr[:], g4_d[m:m + 1, :].bitcast(f32r))
                gb_ps = ps_u.tile([T, F], f32, tag="u")
                for n in range(3):
                    nc.tensor.matmul(
                        gb_ps[:, 512 * n:512 * (n + 1)],
                        ones1[:, :],
                        gr[:, 512 * n:512 * (n + 1)],
                        start=True, stop=True,
                    )
                gb = wpool.tile([T, F], f32, tag=f"Gb{m}")
                nc.scalar.copy(gb[:], gb_ps[:])
                Gb.append(gb)

            woutT_sb = []
            for c in range(12):
                wt = wpool.tile([128, D], f32r, tag=f"woutT{c}")
                nc.sync.dma_start(wt[:], woutT_d[128 * c:128 * (c + 1), :].bitcast(f32r))
                woutT_sb.append(wt)

            # ---------- h0 = x + Sb0 ----------
            h = work.tile([T, D], f32, tag="h", bufs=2)
            nc.vector.tensor_add(h[:], x_sb[:], Sb[0][:])

            accs = {}
            for j in (1, 2, 3):
                accs[j] = apool.tile([T, F], f32, tag=f"acc{j}", name=f"acc{j}")

            # ---------- main loop ----------
            for i in range(NL):
                # hT (stationary for MM1)
                hT_ps = ps_t.tile([T, D], f32, tag="t")
                for k in range(6):
                    nc.tensor.transpose(
                        hT_ps[:, 128 * k:128 * (k + 1)],
                        h[:, 128 * k:128 * (k + 1)],
                        ident[:],
                    )
                hT_sb = work.tile([T, D], f32r, tag="hT_sb", bufs=1)
                nc.scalar.copy(hT_sb[:], hT_ps[:])

                # MM1: u = h @ W_in^T   [T, F]
                u_ps = ps_u.tile([T, F], f32, tag="u")
                for k in range(6):
                    for n in range(3):
                        nc.tensor.matmul(
                            u_ps[:, 512 * n:512 * (n + 1)],
                            hT_sb[:, 128 * k:128 * (k + 1)],
                            winT_sb[k][:, 512 * n:512 * (n + 1)],
                            start=(k == 0), stop=(k == 5),
                        )

                # combine: y = G0*u (+ acc_i)
                y = work.tile([T, F], f32, tag="y", bufs=1)
                if i == 0:
                    for n in range(3):
                        sl = slice(512 * n, 512 * (n + 1))
                        nc.vector.tensor_mul(y[:, sl], u_ps[:, sl], Gb[0][:, sl])
                else:
                    for n in range(3):
                        sl = slice(512 * n, 512 * (n + 1))
                        nc.vector.tensor_mul(y[:, sl], u_ps[:, sl], Gb[0][:, sl])
                        nc.vector.tensor_add(y[:, sl], y[:, sl], accs[i][:, sl])

                # acc updates (off critical path): acc_j += G_{j-i} * u
                for j in range(i + 1, NL):
                    m = j - i
                    if i == 0:
                        nc.vector.tensor_mul(accs[j][:], u_ps[:], Gb[m][:])
                    else:
                        tmp_a = work.tile([T, F], f32, tag="tmp_a", bufs=2)
                        nc.vector.tensor_mul(tmp_a[:], u_ps[:], Gb[m][:])
                        nc.gpsimd.tensor_add(accs[j][:], accs[j][:], tmp_a[:])

                # yT (stationary for MM2)
                yT_ps = ps_t.tile([T, F], f32, tag="t")
                for c in range(12):
                    nc.tensor.transpose(
                        yT_ps[:, 128 * c:128 * (c + 1)],
                        y[:, 128 * c:128 * (c + 1)],
                        ident[:],
                    )
                yT_sb = work.tile([T, F], f32r, tag="yT_sb", bufs=1)
                for n in range(3):
                    sl = slice(512 * n, 512 * (n + 1))
                    nc.scalar.copy(yT_sb[:, sl], yT_ps[:, sl])

                # MM2: z = y @ out_proj^T   [T, D]
                z_ps = ps_z.tile([T, D], f32, tag="z")
                for c in range(12):
                    for off, nn in ((0, 512), (512, 256)):
                        nc.tensor.matmul(
                            z_ps[:, off:off + nn],
                            yT_sb[:, 128 * c:128 * (c + 1)],
                            woutT_sb[c][:, off:off + nn],
                            start=(c == 0), stop=(c == 11),
                        )

                # mixer rmsnorm + residual + loop rmsnorm
                ss_z = scal.tile([T, 1], f32, tag="ss_z")
                sq_scr = work.tile([T, D], f32, tag="scr", bufs=2)
                nc.scalar.activation(sq_scr[:], z_ps[:], AF.Square, accum_out=ss_z[:])
                sq_z = scal.tile([T, 1], f32, tag="sq_z")
                nc.scalar.activation(sq_z[:], ss_z[:], AF.Sqrt,
                                     bias=eps_t[:, :], scale=1.0 / D)
                rs_z = scal.tile([T, 1], f32, tag="rs_z")
                nc.vector.reciprocal(rs_z[:], sq_z[:])

                # w = z * rs_z + h
                w = work.tile([T, D], f32, tag="w", bufs=1)
                nc.vector.scalar_tensor_tensor(
                    out=w[:], in0=z_ps[:], scalar=rs_z[:], in1=h[:],
                    op0=AL.mult, op1=AL.add,
                )

                ss_w = scal.tile([T, 1], f32, tag="ss_w")
                sq_scr2 = work.tile([T, D], f32, tag="scr", bufs=2)
                nc.scalar.activation(sq_scr2[:], w[:], AF.Square, accum_out=ss_w[:])
                sq_w = scal.tile([T, 1], f32, tag="sq_w")
                nc.scalar.activation(sq_w[:], ss_w[:], AF.Sqrt,
                                     bias=eps_t[:, :], scale=1.0 / D)
                rs_w = scal.tile([T, 1], f32, tag="rs_w")
                nc.vector.reciprocal(rs_w[:], sq_w[:])

                if i < NL - 1:
                    h_next = work.tile([T, D], f32, tag="h", bufs=2)
                    nc.vector.scalar_tensor_tensor(
                        out=h_next[:], in0=w[:], scalar=rs_w[:], in1=Sb[i + 1][:],
                        op0=AL.mult, op1=AL.add,
                    )
                    h = h_next
                else:
                    nc.vector.tensor_scalar_mul(w[:], w[:], rs_w[:, :])
                    nc.sync.dma_start(out_d[:, :], w[:])

    nc.compile()
    return nc


def _host_prep(x, in_proj_base, lora_A, lora_B, A_theta, B_real, B_imag,
               C_real, C_imag, out_proj_w, step_emb):
    W_in = in_proj_base.astype(np.float64) + 2.0 * (
        lora_B.astype(np.float64) @ lora_A.astype(np.float64))
    winT = np.ascontiguousarray(W_in.T).astype(np.float32)
    woutT = np.ascontiguousarray(out_proj_w.T).astype(np.float32)

    th = A_theta.astype(np.float64)
    P = (C_real.astype(np.float64) * B_real.astype(np.float64)
         - C_imag.astype(np.float64) * B_imag.astype(np.float64))
    Q = (C_real.astype(np.float64) * B_imag.astype(np.float64)
         + C_imag.astype(np.float64) * B_real.astype(np.float64))
    g4 = np.stack([
        (P * np.cos(m * th) - Q * np.sin(m * th)).sum(-1).reshape(-1)
        for m in range(NL)
    ]).astype(np.float32)                                   # [4, 1536]
    s4 = np.ascontiguousarray(step_emb).astype(np.float32)  # [4, 768]
    return winT, woutT, g4, s4


def kernel(x, in_proj_base, lora_A, lora_B, A_theta, B_real, B_imag,
           C_real, C_imag, out_proj_w, mixer_norm_w, loop_norm_w, step_emb,
           _trace=False):
    x = np.asarray(x, dtype=np.float32)
    winT, woutT, g4, s4 = _host_prep(
        np.asarray(x), np.asarray(in_proj_base), np.asarray(lora_A),
        np.asarray(lora_B), np.asarray(A_theta), np.asarray(B_real),
        np.asarray(B_imag), np.asarray(C_real), np.asarray(C_imag),
        np.asarray(out_proj_w), np.asarray(step_emb))
    # mixer_norm_w / loop_norm_w are ones per the problem spec; rmsnorm weight
    # multiplies are identity and omitted on device.

    if "nc" not in _CACHE:
        _CACHE["nc"] = build_nc()
    nc = _CACHE["nc"]

    shared = {"winT": winT, "woutT": woutT, "g4": g4, "s4": s4}
    in_maps = [
        {**shared, "x_in": np.ascontiguousarray(x[0, T * c:T * (c + 1), :])}
        for c in range(NCORES)
    ]
    res = run_bass_kernel_spmd(nc, in_maps, list(range(NCORES)), trace=_trace)
    out = np.concatenate(
        [np.asarray(res.results[c]["x_out"]) for c in range(NCORES)], axis=0)
    if _trace:
        _CACHE["last_result"] = res
    return out[None, :, :].astype(np.float32)



# revision 10
# speedup vs baseline: 1.1878x; 1.1878x over previous
"""Trainium2 Bass kernel for RecursiveMamba130M (fp16 pipelined version).

Math: the complex SSM state never needs materializing. With
  R = cos(theta) + j sin(theta),  Bc = Br + j Bi,  Cc = Cr + j Ci,
the per-loop output collapses to
  y_i[t, f] = sum_{k<=i} G_{i-k}[f] * u_k[t, f],   u_k = h_k @ W_in^T
where G_m[f] = sum_s (CrBr - CiBi) cos(m th) - (CrBi + CiBr) sin(m th).

Sharding: data-parallel over the 1024 sequence positions (128 tokens
per core, no collectives); small weights replicated.

Per-core device program (tokens on partitions, fp16 data / fp32 PSUM):
  loop i in 0..3:
    [DVE during MM1] hist = sum_{k<i} G_{i-k} * u_k   (from fp16 u history)
    u    = h @ W_in^T               (PE fp16, n-outer, N=512 tiles)
    y    = G0*u + hist              (DVE)
    yT   = PE-transpose(y)          (12x 128x128 fp16)
    z    = y @ out_proj^T           (PE fp16)
    rmsnorm chain via the identity
      sum w^2 = rs_z*(rs_z*sum z^2 + 2*sum z*h) + sum h^2
    h' = (z*rs_z + h)*rs_w + step_emb[i+1]
    hT   = PE-transpose(h')         (6x 128x128 fp16)
h0/h0T/sum h0^2 are precomputed on host so loop 0 starts immediately.
"""

import numpy as np

import concourse.bass as bass
import concourse.tile as tile
from concourse.bacc import Bacc
from concourse import masks, mybir
from concourse.bass_utils import run_bass_kernel_spmd

T = 128          # tokens per core
D = 768          # d_model
F = 1536         # 2 * d_model
NL = 4           # reasoning loops
NCORES = 8
EPS = 1e-6

f16 = mybir.dt.float16
f32 = mybir.dt.float32
AL = mybir.AluOpType
AF = mybir.ActivationFunctionType

_CACHE = {}


def build_nc():
    nc = Bacc()
    h0_d = nc.dram_tensor("h0", [T, D], f16, kind="ExternalInput")
    h0T_d = nc.dram_tensor("h0T", [D, T], f16, kind="ExternalInput")
    winT_d = nc.dram_tensor("winT", [D, F], f16, kind="ExternalInput")
    woutT_d = nc.dram_tensor("woutT", [F, D], f16, kind="ExternalInput")
    g4_d = nc.dram_tensor("g4", [NL, F], f16, kind="ExternalInput")
    s4_d = nc.dram_tensor("s4", [NL, D], f16, kind="ExternalInput")
    out_d = nc.dram_tensor("x_out", [T, D], f32, kind="ExternalOutput")

    with tile.TileContext(nc) as tc:
        with (
            tc.tile_pool(name="wpool", bufs=1) as wpool,
            tc.tile_pool(name="hist", bufs=1) as hpool,
            tc.tile_pool(name="work", bufs=2) as work,
            tc.tile_pool(name="scal", bufs=1) as scal,
            tc.tile_pool(name="ps_t", bufs=1, space="PSUM") as ps_t,
            tc.tile_pool(name="ps_u", bufs=1, space="PSUM") as ps_u,
            tc.tile_pool(name="ps_z", bufs=1, space="PSUM") as ps_z,
        ):
            # ---------- constants ----------
            ident = wpool.tile([128, 128], f16, tag="ident")
            masks.make_identity(nc, ident[:])
            ones1 = wpool.tile([1, 128], f16, tag="ones1")
            nc.vector.memset(ones1[:], 1.0)
            eps_t = wpool.tile([T, 1], f32, tag="eps_t")
            nc.vector.memset(eps_t[:], EPS)

            # ---------- small DMAs first (sync queue) ----------
            g_rows = []
            for m in range(NL):
                gr = work.tile([1, F], f16, tag="g_row", bufs=4, name=f"g_row{m}")
                nc.sync.dma_start(gr[:], g4_d[m:m + 1, :])
                g_rows.append(gr)
            s_rows = []
            for i in range(1, NL):
                sr = work.tile([1, D], f16, tag="s_row", bufs=3, name=f"s_row{i}")
                nc.sync.dma_start(sr[:], s4_d[i:i + 1, :])
                s_rows.append(sr)
            h = work.tile([T, D], f16, tag="h", bufs=2)
            nc.sync.dma_start(h[:], h0_d[:, :])
            hT_sb = work.tile([T, D], f16, tag="hT_sb", bufs=2)
            for k in range(6):
                nc.sync.dma_start(hT_sb[:, 128 * k:128 * (k + 1)],
                                  h0T_d[128 * k:128 * (k + 1), :])

            # winT chunks on the sync queue (needed progressively by MM1)
            winT_sb = []
            for k in range(6):
                wt = wpool.tile([128, F], f16, tag=f"winT{k}")
                nc.sync.dma_start(wt[:], winT_d[128 * k:128 * (k + 1), :])
                winT_sb.append(wt)

            # woutT chunks after winT on the same sync queue: keeps winT at
            # full HBM bandwidth (loop-0 MM1 is paced by winT arrival)
            woutT_sb = []
            for c in range(12):
                wt = wpool.tile([128, D], f16, tag=f"woutT{c}")
                nc.sync.dma_start(wt[:], woutT_d[128 * c:128 * (c + 1), :])
                woutT_sb.append(wt)

            # ---------- G / step_emb broadcast tiles (PE ones-trick) ----------
            Gb = []
            for m in range(NL):
                gb_ps = ps_u.tile([T, F], f32, tag="u")
                for n in range(3):
                    nc.tensor.matmul(
                        gb_ps[:, 512 * n:512 * (n + 1)],
                        ones1[:, :],
                        g_rows[m][:, 512 * n:512 * (n + 1)],
                        start=True, stop=True,
                    )
                gb = wpool.tile([T, F], f16, tag=f"Gb{m}")
                nc.scalar.copy(gb[:], gb_ps[:])
                Gb.append(gb)

            Sb = {}
            for i in range(1, NL):
                sb_ps = ps_z.tile([T, D], f32, tag="z")
                for off, nn in ((0, 512), (512, 256)):
                    nc.tensor.matmul(
                        sb_ps[:, off:off + nn],
                        ones1[:, :],
                        s_rows[i - 1][:, off:off + nn],
                        start=True, stop=True,
                    )
                sb = wpool.tile([T, D], f16, tag=f"Sb{i}")
                nc.scalar.copy(sb[:], sb_ps[:])
                Sb[i] = sb

            u_hist = [
                hpool.tile([T, F], f16, tag=f"u_h{k}", name=f"u_h{k}")
                for k in range(3)
            ]

            # ---------- main loop ----------
            for i in range(NL):
                # history combine on DVE, overlapped with MM1
                if i >= 1:
                    acc = work.tile([T, F], f16, tag="acc", bufs=2)
                    nc.vector.tensor_mul(acc[:], u_hist[i - 1][:], Gb[1][:])
                    for back in range(2, i + 1):
                        tmp_a = work.tile([T, F], f16, tag="tmp_a", bufs=2)
                        nc.vector.tensor_mul(
                            tmp_a[:], u_hist[i - back][:], Gb[back][:])
                        nc.vector.tensor_add(acc[:], acc[:], tmp_a[:])

                # MM1: u = h @ W_in^T  [T, F].
                # Loop 0: k-outer (consumes winT chunks in DMA arrival order).
                # Later loops: n-outer so u slices finish early for the DVE.
                u_ps = ps_u.tile([T, F], f32, tag="u")
                if i == 0:
                    for k in range(6):
                        for n in range(3):
                            sl = slice(512 * n, 512 * (n + 1))
                            nc.tensor.matmul(
                                u_ps[:, sl],
                                hT_sb[:, 128 * k:128 * (k + 1)],
                                winT_sb[k][:, sl],
                                start=(k == 0), stop=(k == 5),
                            )
                else:
                    for n in range(3):
                        sl = slice(512 * n, 512 * (n + 1))
                        for k in range(6):
                            nc.tensor.matmul(
                                u_ps[:, sl],
                                hT_sb[:, 128 * k:128 * (k + 1)],
                                winT_sb[k][:, sl],
                                start=(k == 0), stop=(k == 5),
                            )

                # y = G0*u (+ hist); u_sb fp16 copy for future history
                y = work.tile([T, F], f16, tag="y", bufs=1)
                for n in range(3):
                    sl = slice(512 * n, 512 * (n + 1))
                    nc.vector.tensor_mul(y[:, sl], u_ps[:, sl], Gb[0][:, sl])
                    if i >= 1:
                        nc.vector.tensor_add(y[:, sl], y[:, sl], acc[:, sl])
                    if i < 3:
                        nc.scalar.copy(u_hist[i][:, sl], u_ps[:, sl])

                # yT (stationary for MM2)
                yT_ps = ps_t.tile([T, F], f16, tag="t")
                for c in range(12):
                    nc.tensor.transpose(
                        yT_ps[:, 128 * c:128 * (c + 1)],
                        y[:, 128 * c:128 * (c + 1)],
                        ident[:],
                    )
                yT_sb = work.tile([T, F], f16, tag="yT_sb", bufs=1)
                nc.vector.tensor_copy(yT_sb[:, 0:512], yT_ps[:, 0:512])
                nc.scalar.copy(yT_sb[:, 512:1024], yT_ps[:, 512:1024])
                nc.scalar.copy(yT_sb[:, 1024:1536], yT_ps[:, 1024:1536])

                # MM2: z = y @ out_proj^T   [T, D]
                z_ps = ps_z.tile([T, D], f32, tag="z")
                for c in range(12):
                    for off, nn in ((0, 512), (512, 256)):
                        nc.tensor.matmul(
                            z_ps[:, off:off + nn],
                            yT_sb[:, 128 * c:128 * (c + 1)],
                            woutT_sb[c][:, off:off + nn],
                            start=(c == 0), stop=(c == 11),
                        )

                # ---- mixer rmsnorm + residual + loop rmsnorm ----
                ss_z = scal.tile([T, 1], f32, tag="ss_z", bufs=2)
                sq_scr = work.tile([T, D], f16, tag="scr", bufs=2)
                nc.scalar.activation(sq_scr[:], z_ps[:], AF.Square,
                                     accum_out=ss_z[:])
                sq_z = scal.tile([T, 1], f32, tag="sq_z", bufs=2)
                nc.scalar.activation(sq_z[:], ss_z[:], AF.Sqrt,
                                     bias=eps_t[:, :], scale=1.0 / D)
                rs_z = scal.tile([T, 1], f32, tag="rs_z", bufs=2)
                nc.vector.reciprocal(rs_z[:], sq_z[:])

                # w = z*rs_z + h
                last = i == NL - 1
                w = work.tile([T, D], f32 if last else f16, tag="w", bufs=1,
                              name=f"w{i}")
                nc.vector.scalar_tensor_tensor(
                    out=w[:], in0=z_ps[:], scalar=rs_z[:, :], in1=h[:],
                    op0=AL.mult, op1=AL.add)

                ss_w = scal.tile([T, 1], f32, tag="ss_w", bufs=2)
                sq_scr2 = work.tile([T, D], f16, tag="scr", bufs=2)
                nc.scalar.activation(sq_scr2[:], w[:], AF.Square,
                                     accum_out=ss_w[:])
                sq_w = scal.tile([T, 1], f32, tag="sq_w", bufs=2)
                nc.scalar.activation(sq_w[:], ss_w[:], AF.Sqrt,
                                     bias=eps_t[:, :], scale=1.0 / D)
                rs_w = scal.tile([T, 1], f32, tag="rs_w", bufs=2)
                nc.vector.reciprocal(rs_w[:], sq_w[:])

                if not last:
                    h_next = work.tile([T, D], f16, tag="h", bufs=2)
                    nc.vector.scalar_tensor_tensor(
                        out=h_next[:], in0=w[:], scalar=rs_w[:, :],
                        in1=Sb[i + 1][:], op0=AL.mult, op1=AL.add)

                    # hT for next MM1
                    hT_ps = ps_t.tile([T, D], f16, tag="t")
                    for k in range(6):
                        nc.tensor.transpose(
                            hT_ps[:, 128 * k:128 * (k + 1)],
                            h_next[:, 128 * k:128 * (k + 1)],
                            ident[:],
                        )
                    hT_next = work.tile([T, D], f16, tag="hT_sb", bufs=2)
                    nc.vector.tensor_copy(hT_next[:], hT_ps[:])

                    h = h_next
                    hT_sb = hT_next
                else:
                    x_out = work.tile([T, D], f32, tag="x_out", bufs=1)
                    nc.vector.tensor_scalar_mul(x_out[:], w[:], rs_w[:, :])
                    nc.sync.dma_start(out_d[:, :], x_out[:])

    nc.compile()
    return nc


def _host_prep(x, in_proj_base, lora_A, lora_B, A_theta, B_real, B_imag,
               C_real, C_imag, out_proj_w, step_emb):
    W_in = in_proj_base.astype(np.float64) + 2.0 * (
        lora_B.astype(np.float64) @ lora_A.astype(np.float64))
    winT = np.ascontiguousarray(W_in.T).astype(np.float16)
    woutT = np.ascontiguousarray(out_proj_w.T).astype(np.float16)

    th = A_theta.astype(np.float64)
    P = (C_real.astype(np.float64) * B_real.astype(np.float64)
         - C_imag.astype(np.float64) * B_imag.astype(np.float64))
    Q = (C_real.astype(np.float64) * B_imag.astype(np.float64)
         + C_imag.astype(np.float64) * B_real.astype(np.float64))
    g4 = np.stack([
        (P * np.cos(m * th) - Q * np.sin(m * th)).sum(-1).reshape(-1)
        for m in range(NL)
    ]).astype(np.float16)                                   # [4, 1536]
    s4 = np.ascontiguousarray(step_emb).astype(np.float16)  # [4, 768]

    h0 = (x[0].astype(np.float64) + step_emb[0].astype(np.float64)
          ).astype(np.float16)                              # [1024, 768]
    h0T = np.ascontiguousarray(h0.T)                        # [768, 1024]
    ssh0 = (h0.astype(np.float32) ** 2).sum(-1, keepdims=True)  # [1024, 1]
    return winT, woutT, g4, s4, h0, h0T, ssh0


def kernel(x, in_proj_base, lora_A, lora_B, A_theta, B_real, B_imag,
           C_real, C_imag, out_proj_w, mixer_norm_w, loop_norm_w, step_emb,
           _trace=False):
    winT, woutT, g4, s4, h0, h0T, ssh0 = _host_prep(
        np.asarray(x, dtype=np.float32), np.asarray(in_proj_base),
        np.asarray(lora_A), np.asarray(lora_B), np.asarray(A_theta),
        np.asarray(B_real), np.asarray(B_imag), np.asarray(C_real),
        np.asarray(C_imag), np.asarray(out_proj_w), np.asarray(step_emb))
    # mixer_norm_w / loop_norm_w are ones per the problem spec; rmsnorm weight
    # multiplies are identity and omitted on device.

    if "nc" not in _CACHE:
        _CACHE["nc"] = build_nc()
    nc = _CACHE["nc"]

    shared = {"winT": winT, "woutT": woutT, "g4": g4, "s4": s4}
    in_maps = [
        {**shared,
         "h0": np.ascontiguousarray(h0[T * c:T * (c + 1), :]),
         "h0T": np.ascontiguousarray(h0T[:, T * c:T * (c + 1)])}
        for c in range(NCORES)
    ]
    res = run_bass_kernel_spmd(nc, in_maps, list(range(NCORES)), trace=_trace)
    out = np.concatenate(
        [np.asarray(res.results[c]["x_out"]) for c in range(NCORES)], axis=0)
    if _trace:
        _CACHE["last_result"] = res
    return out[None, :, :].astype(np.float32)


# revision 14
# speedup vs baseline: 1.2744x; 1.0730x over previous
"""Trainium2 Bass kernel for RecursiveMamba130M (fp16, v-recursion pipeline).

Math: the complex SSM state collapses to a depthwise convolution over
reasoning loops:
  y_i[t, f] = sum_{k<=i} G_{i-k}[f] * u_k[t, f],   u_k = h_k @ W_in^T
  G_m[f] = sum_s (CrBr - CiBi) cos(m th) - (CrBi + CiBr) sin(m th)

Key restructure: h_{i+1} = a*z_i + b*h_i + step_{i+1} with per-token
scalars a = rs_z*rs_w, b = rs_w.  Right-multiplying by W_in^T:
  u_{i+1} = a*(z_i @ W_in^T) + b*u_i + su_{i+1},   su = step @ W_in^T
so MM1 of loop i+1 becomes v = z_i @ W_in^T, which depends only on z_i
(NOT on the rmsnorm scalars).  The PE therefore flows
MM2 -> transpose(z) -> MM1' -> transpose(y) -> MM2 with no norm stall;
the rmsnorm chain, u/h recovery and acc updates overlap under MM1'/MM2.

rmsnorm sums via the identity
  sum w^2 = rs_z*(rs_z*sum z^2 + 2*sum z*h) + sum h^2.

Sharding: data-parallel over sequence: 128 tokens per core, no
collectives; weights replicated.  fp16 data / fp32 PSUM + scalars.
"""

import numpy as np

import concourse.bass as bass
import concourse.tile as tile
from concourse.bacc import Bacc
from concourse import masks, mybir
from concourse.bass_utils import run_bass_kernel_spmd

T = 128          # tokens per core
D = 768          # d_model
F = 1536         # 2 * d_model
NL = 4           # reasoning loops
NCORES = 8
EPS = 1e-6

f16 = mybir.dt.float16
f32 = mybir.dt.float32
AL = mybir.AluOpType
AF = mybir.ActivationFunctionType

_CACHE = {}


def build_nc():
    nc = Bacc()
    h0_d = nc.dram_tensor("h0", [T, D], f16, kind="ExternalInput")
    h0T_d = nc.dram_tensor("h0T", [D, T], f16, kind="ExternalInput")
    ssh0_d = nc.dram_tensor("ssh0", [T, 1], f32, kind="ExternalInput")
    winT_d = nc.dram_tensor("winT", [D, F], f16, kind="ExternalInput")
    woutT_d = nc.dram_tensor("woutT", [F, D], f16, kind="ExternalInput")
    g4_d = nc.dram_tensor("g4", [NL, F], f16, kind="ExternalInput")
    s4_d = nc.dram_tensor("s4", [NL, D], f16, kind="ExternalInput")
    su4_d = nc.dram_tensor("su4", [NL, F], f16, kind="ExternalInput")
    out_d = nc.dram_tensor("x_out", [T, D], f32, kind="ExternalOutput")

    with tile.TileContext(nc) as tc:
        with (
            tc.tile_pool(name="wpool", bufs=1) as wpool,
            tc.tile_pool(name="apool", bufs=1) as apool,
            tc.tile_pool(name="work", bufs=2) as work,
            tc.tile_pool(name="scal", bufs=1) as scal,
            tc.tile_pool(name="ps_t", bufs=1, space="PSUM") as ps_t,
            tc.tile_pool(name="ps_v", bufs=1, space="PSUM") as ps_v,
            tc.tile_pool(name="ps_z", bufs=1, space="PSUM") as ps_z,
        ):
            # ---------- constants ----------
            ident = wpool.tile([128, 128], f16, tag="ident")
            masks.make_identity(nc, ident[:])
            ones1 = wpool.tile([1, 128], f16, tag="ones1")
            nc.vector.memset(ones1[:], 1.0)
            eps_t = wpool.tile([T, 1], f32, tag="eps_t")
            nc.vector.memset(eps_t[:], EPS)

            # ---------- small DMAs first (sync queue) ----------
            g_rows = []
            for m in range(NL):
                gr = work.tile([1, F], f16, tag="g_row", bufs=4, name=f"g_row{m}")
                nc.sync.dma_start(gr[:], g4_d[m:m + 1, :])
                g_rows.append(gr)
            s_rows = {}
            for i in range(1, NL):
                sr = work.tile([1, D], f16, tag="s_row", bufs=3, name=f"s_row{i}")
                nc.sync.dma_start(sr[:], s4_d[i:i + 1, :])
                s_rows[i] = sr
            su_rows = {}
            for i in range(1, NL):
                sr = work.tile([1, F], f16, tag="su_row", bufs=3, name=f"su_row{i}")
                nc.sync.dma_start(sr[:], su4_d[i:i + 1, :])
                su_rows[i] = sr
            ssh0 = scal.tile([T, 1], f32, tag="ssh0")
            nc.sync.dma_start(ssh0[:], ssh0_d[:, :])
            h = work.tile([T, D], f16, tag="h", bufs=2)
            nc.sync.dma_start(h[:], h0_d[:, :])
            hT0 = work.tile([T, D], f16, tag="hT0", bufs=1)
            for k in range(6):
                nc.sync.dma_start(hT0[:, 128 * k:128 * (k + 1)],
                                  h0T_d[128 * k:128 * (k + 1), :])

            winT_sb = []
            for k in range(6):
                wt = wpool.tile([128, F], f16, tag=f"winT{k}")
                nc.sync.dma_start(wt[:], winT_d[128 * k:128 * (k + 1), :])
                winT_sb.append(wt)
            woutT_sb = []
            for c in range(12):
                wt = wpool.tile([128, D], f16, tag=f"woutT{c}")
                nc.sync.dma_start(wt[:], woutT_d[128 * c:128 * (c + 1), :])
                woutT_sb.append(wt)

            SL = [slice(512 * n, 512 * (n + 1)) for n in range(3)]

            # ---------- broadcast tiles via PE ones-trick ----------
            def bcast_f(row, tag):  # [1, F] -> [128, F]
                sb = wpool.tile([T, F], f16, tag=tag)
                for n in range(3):
                    ps = ps_v.tile([T, 512], f32, tag=f"v{n}", name=f"bc_{tag}{n}")
                    nc.tensor.matmul(ps[:, :], ones1[:, :], row[:, SL[n]],
                                     start=True, stop=True)
                    nc.scalar.copy(sb[:, SL[n]], ps[:, :])
                return sb

            def bcast_d(row, tag):  # [1, D] -> [128, D]
                ps = ps_z.tile([T, D], f32, tag="z")
                for off, nn in ((0, 512), (512, 256)):
                    nc.tensor.matmul(ps[:, off:off + nn], ones1[:, :],
                                     row[:, off:off + nn], start=True, stop=True)
                sb = wpool.tile([T, D], f16, tag=tag)
                nc.scalar.copy(sb[:], ps[:])
                return sb

            Gb = [bcast_f(g_rows[m], f"Gb{m}") for m in range(NL)]
            SU = {i: bcast_f(su_rows[i], f"SU{i}") for i in range(1, NL)}
            Sb = {i: bcast_d(s_rows[i], f"Sb{i}") for i in range(1, NL)}

            accs = {j: apool.tile([T, F], f16, tag=f"acc{j}", name=f"acc{j}")
                    for j in (1, 2, 3)}
            ss_h = ssh0
            rs_w_p = a_p = None
            u_prev = None
            zT_sb = None

            for i in range(NL):
                first, last = i == 0, i == NL - 1

                # ---- T1_n = b*u_prev + SU_i (early window ops, old scalars)
                t1s = []
                if not first:
                    for n in range(3):
                        t1 = work.tile([T, 512], f16, tag=f"t1_{n}", bufs=2,
                                       name=f"t1_{n}")
                        nc.vector.scalar_tensor_tensor(
                            out=t1[:], in0=u_prev[:, SL[n]],
                            scalar=rs_w_p[:, :], in1=SU[i][:, SL[n]],
                            op0=AL.mult, op1=AL.add)
                        t1s.append(t1)

                # ---- MM1: v_i = (h_0 or z_{i-1}) @ W_in^T ----
                v_ps = [ps_v.tile([T, 512], f32, tag=f"v{n}", name=f"v{n}_{i}")
                        for n in range(3)]
                lhsT = hT0 if first else zT_sb
                if first:
                    for k in range(6):      # k-outer: winT DMA arrival order
                        for n in range(3):
                            nc.tensor.matmul(
                                v_ps[n][:, :], lhsT[:, 128 * k:128 * (k + 1)],
                                winT_sb[k][:, SL[n]],
                                start=(k == 0), stop=(k == 5))
                else:
                    for n in range(3):      # n-outer: slices finish early
                        for k in range(6):
                            nc.tensor.matmul(
                                v_ps[n][:, :], lhsT[:, 128 * k:128 * (k + 1)],
                                winT_sb[k][:, SL[n]],
                                start=(k == 0), stop=(k == 5))

                # ---- per-slice: u and y; then yT transposes ----
                u_cur = work.tile([T, F], f16, tag="u", bufs=2, name=f"u{i}")
                y = work.tile([T, F], f16, tag="y", bufs=1)
                yT_ps = ps_t.tile([T, F], f16, tag="t")
                for n in range(3):
                    sl = SL[n]
                    if first:
                        # u = v (copy for acc updates, ACT); y = G0*u
                        nc.scalar.copy(u_cur[:, sl], v_ps[n][:, :])
                        nc.vector.tensor_mul(y[:, sl], v_ps[n][:, :],
                                             Gb[0][:, sl])
                    else:
                        # u = a*v + T1   (critical)
                        nc.vector.scalar_tensor_tensor(
                            out=u_cur[:, sl], in0=v_ps[n][:, :],
                            scalar=a_p[:, :], in1=t1s[n][:],
                            op0=AL.mult, op1=AL.add)
                        # y = G0*u + acc_i
                        ym = work.tile([T, 512], f16, tag=f"ym_{n}", bufs=2,
                                       name=f"ym_{n}")
                        nc.vector.tensor_mul(ym[:], u_cur[:, sl], Gb[0][:, sl])
                        nc.vector.tensor_add(y[:, sl], ym[:], accs[i][:, sl])
                    for c in range(4 * n, 4 * n + 4):
                        nc.tensor.transpose(
                            yT_ps[:, 128 * c:128 * (c + 1)],
                            y[:, 128 * c:128 * (c + 1)], ident[:])

                yT_sb = work.tile([T, F], f16, tag="yT_sb", bufs=1)
                for n in range(3):
                    nc.scalar.copy(yT_sb[:, SL[n]], yT_ps[:, SL[n]])

                # ---- eager acc updates for future loops (off-path) ----
                for j in range(i + 1, NL):
                    m = j - i
                    if first:
                        nc.vector.tensor_mul(accs[j][:], u_cur[:], Gb[m][:])
                    else:
                        tmp_a = work.tile([T, F], f16, tag="tmp_a", bufs=2)
                        nc.vector.tensor_mul(tmp_a[:], u_cur[:], Gb[m][:])
                        nc.gpsimd.tensor_add(accs[j][:], accs[j][:], tmp_a[:])

                # ---- MM2: z = y @ out_proj^T ----
                z_ps = ps_z.tile([T, D], f32, tag="z")
                for c in range(12):
                    for off, nn in ((0, 512), (512, 256)):
                        nc.tensor.matmul(
                            z_ps[:, off:off + nn],
                            yT_sb[:, 128 * c:128 * (c + 1)],
                            woutT_sb[c][:, off:off + nn],
                            start=(c == 0), stop=(c == 11))

                # ---- z -> SBUF f16; zT (next loop's MM1 stationary) ----
                z_sb = work.tile([T, D], f16, tag="z_sb", bufs=2)
                nc.scalar.copy(z_sb[:, 0:384], z_ps[:, 0:384])
                nc.scalar.copy(z_sb[:, 384:768], z_ps[:, 384:768])
                if not last:
                    zT_ps = ps_t.tile([T, D], f16, tag="t")
                    for k in range(6):
                        nc.tensor.transpose(
                            zT_ps[:, 128 * k:128 * (k + 1)],
                            z_sb[:, 128 * k:128 * (k + 1)], ident[:])
                    zT_sb = work.tile([T, D], f16, tag="zT_sb", bufs=2)
                    nc.scalar.copy(zT_sb[:, 0:384], zT_ps[:, 0:384])
                    nc.scalar.copy(zT_sb[:, 384:768], zT_ps[:, 384:768])

                # ---- rmsnorm scalars via the identity (loose deadlines) ----
                ss_z = scal.tile([T, 1], f32, tag="ss_z", bufs=2)
                sq_scr = work.tile([T, D], f16, tag="scr", bufs=3)
                nc.scalar.activation(sq_scr[:], z_ps[:], AF.Square,
                                     accum_out=ss_z[:])
                szh2 = scal.tile([T, 1], f32, tag="szh2", bufs=2)
                zh_scr = work.tile([T, D], f16, tag="scr", bufs=3)
                nc.vector.scalar_tensor_tensor(
                    out=zh_scr[:], in0=z_sb[:], scalar=2.0, in1=h[:],
                    op0=AL.mult, op1=AL.mult, accum_out=szh2[:])

                sq_z = scal.tile([T, 1], f32, tag="sq_z", bufs=2)
                nc.scalar.activation(sq_z[:], ss_z[:], AF.Sqrt,
                                     bias=eps_t[:, :], scale=1.0 / D)
                rs_z = scal.tile([T, 1], f32, tag="rs_z", bufs=2)
                nc.vector.reciprocal(rs_z[:], sq_z[:])

                q_t = scal.tile([T, 1], f32, tag="q_t", bufs=2)
                nc.vector.scalar_tensor_tensor(
                    out=q_t[:], in0=ss_z[:], scalar=rs_z[:, :], in1=szh2[:],
                    op0=AL.mult, op1=AL.add)
                ss_w = scal.tile([T, 1], f32, tag="ss_w", bufs=2)
                nc.vector.scalar_tensor_tensor(
                    out=ss_w[:], in0=q_t[:], scalar=rs_z[:, :], in1=ss_h[:],
                    op0=AL.mult, op1=AL.add)
                sq_w = scal.tile([T, 1], f32, tag="sq_w", bufs=2)
                nc.scalar.activation(sq_w[:], ss_w[:], AF.Sqrt,
                                     bias=eps_t[:, :], scale=1.0 / D)
                rs_w = scal.tile([T, 1], f32, tag="rs_w", bufs=2)
                nc.vector.reciprocal(rs_w[:], sq_w[:])
                a_t = scal.tile([T, 1], f32, tag="a_t", bufs=2)
                nc.vector.tensor_mul(a_t[:], rs_z[:], rs_w[:])

                # ---- h update: h' = a*z + (b*h + Sb_{i+1}) ----
                if not last:
                    t2 = work.tile([T, D], f16, tag="t2", bufs=2)
                    nc.vector.scalar_tensor_tensor(
                        out=t2[:], in0=h[:], scalar=rs_w[:, :],
                        in1=Sb[i + 1][:], op0=AL.mult, op1=AL.add)
                    h_next = work.tile([T, D], f16, tag="h", bufs=2)
                    nc.vector.scalar_tensor_tensor(
                        out=h_next[:], in0=z_sb[:], scalar=a_t[:, :],
                        in1=t2[:], op0=AL.mult, op1=AL.add)
                    # ss_h for next loop's identity
                    ss_h_next = scal.tile([T, 1], f32, tag="ss_h", bufs=2)
                    ssh_scr = work.tile([T, D], f16, tag="scr", bufs=3)
                    nc.scalar.activation(ssh_scr[:], h_next[:], AF.Square,
                                         accum_out=ss_h_next[:])
                    h = h_next
                    ss_h = ss_h_next
                else:
                    t2 = work.tile([T, D], f32, tag="t2f", bufs=1)
                    nc.vector.tensor_scalar_mul(t2[:], h[:], rs_w[:, :])
                    x_out = work.tile([T, D], f32, tag="x_out", bufs=1)
                    nc.vector.scalar_tensor_tensor(
                        out=x_out[:], in0=z_ps[:], scalar=a_t[:, :],
                        in1=t2[:], op0=AL.mult, op1=AL.add)
                    nc.sync.dma_start(out_d[:, :], x_out[:])

                rs_w_p, a_p = rs_w, a_t
                u_prev = u_cur

    nc.compile()
    return nc


def _host_prep(x, in_proj_base, lora_A, lora_B, A_theta, B_real, B_imag,
               C_real, C_imag, out_proj_w, step_emb):
    W_in = in_proj_base.astype(np.float64) + 2.0 * (
        lora_B.astype(np.float64) @ lora_A.astype(np.float64))
    winT = np.ascontiguousarray(W_in.T).astype(np.float16)
    woutT = np.ascontiguousarray(out_proj_w.T).astype(np.float16)

    th = A_theta.astype(np.float64)
    P = (C_real.astype(np.float64) * B_real.astype(np.float64)
         - C_imag.astype(np.float64) * B_imag.astype(np.float64))
    Q = (C_real.astype(np.float64) * B_imag.astype(np.float64)
         + C_imag.astype(np.float64) * B_real.astype(np.float64))
    g4 = np.stack([
        (P * np.cos(m * th) - Q * np.sin(m * th)).sum(-1).reshape(-1)
        for m in range(NL)
    ]).astype(np.float16)                                   # [4, 1536]
    s4 = np.ascontiguousarray(step_emb).astype(np.float16)  # [4, 768]
    su4 = (step_emb.astype(np.float64) @ W_in.T).astype(np.float16)  # [4,1536]

    h0 = (x[0].astype(np.float64) + step_emb[0].astype(np.float64)
          ).astype(np.float16)                              # [1024, 768]
    h0T = np.ascontiguousarray(h0.T)                        # [768, 1024]
    ssh0 = (h0.astype(np.float32) ** 2).sum(-1, keepdims=True)  # [1024, 1]
    return winT, woutT, g4, s4, su4, h0, h0T, ssh0


def kernel(x, in_proj_base, lora_A, lora_B, A_theta, B_real, B_imag,
           C_real, C_imag, out_proj_w, mixer_norm_w, loop_norm_w, step_emb,
           _trace=False):
    winT, woutT, g4, s4, su4, h0, h0T, ssh0 = _host_prep(
        np.asarray(x, dtype=np.float32), np.asarray(in_proj_base),
        np.asarray(lora_A), np.asarray(lora_B), np.asarray(A_theta),
        np.asarray(B_real), np.asarray(B_imag), np.asarray(C_real),
        np.asarray(C_imag), np.asarray(out_proj_w), np.asarray(step_emb))
    # mixer_norm_w / loop_norm_w are ones per the problem spec; rmsnorm weight
    # multiplies are identity and omitted on device.

    if "nc" not in _CACHE:
        _CACHE["nc"] = build_nc()
    nc = _CACHE["nc"]

    shared = {"winT": winT, "woutT": woutT, "g4": g4, "s4": s4, "su4": su4}
    in_maps = [
        {**shared,
         "h0": np.ascontiguousarray(h0[T * c:T * (c + 1), :]),
         "h0T": np.ascontiguousarray(h0T[:, T * c:T * (c + 1)]),
         "ssh0": np.ascontiguousarray(ssh0[T * c:T * (c + 1), :])}
        for c in range(NCORES)
    ]
    res = run_bass_kernel_spmd(nc, in_maps, list(range(NCORES)), trace=_trace)
    out = np.concatenate(
        [np.asarray(res.results[c]["x_out"]) for c in range(NCORES)], axis=0)
    if _trace:
        _CACHE["last_result"] = res
    return out[None, :, :].astype(np.float32)
